# revision 32
# baseline (speedup 1.0000x reference)
"""DenoiseNet loss kernel for 8 Trainium2 NeuronCores.

Data parallel over batch (4/core). Exact global BatchNorm via per-layer
AllReduce of (sum, sumsq); BN+ReLU applied as one affine relu(a*h+c).
Big MLP layers run fp8 DoubleRow (2 contract rows/cycle). KNN d'=2x.r-|r|^2
via fp16 hi/lo matmuls row-packed 2-wide with tile_position; argmax by
reduce-max; loss extracted by a one-hot contraction matmul
T=sum_q onehot*sw*[-2x_new^T;1] then sum_r T*[r;|r|^2]. KNN work fills
the AllReduce latency gaps.
"""
import numpy as np

import concourse.bass as bass
import concourse.mybir as mybir
import concourse.tile as tile
from concourse import bacc
from concourse.bass_utils import run_bass_kernel_spmd

dt = mybir.dt
F32 = dt.float32
F16 = dt.float16
F8 = dt.float8e4
AF = mybir.ActivationFunctionType
OP = mybir.AluOpType
AX = mybir.AxisListType
PM = mybir.MatmulPerfMode

B, N, NCORES = 32, 1000, 8
BL = B // NCORES            # 4 batches per core
PTS = BL * N                # 4000 points per core
NITER = 4
NPTS_GLOBAL = B * N         # 32000 (BN population)
EPS = 1e-5
NOISE_DECAY = 4.0
QT = 125                    # q tile (8 per batch)
RP = 1024                   # padded ref points (24 sentinels)
PT = 500                    # pts tile for MLP
NPT = PTS // PT             # 8

LAYERS = [(3, 64, 1), (64, 128, 1), (128, 256, 1), (256, 512, 1),
          (512, 1024, 1), (1024, 512, 1), (512, 256, 1), (256, 3, 0)]
NCI = [max(1, ci // 128) for ci, co, _ in LAYERS]
NCO = [max(1, (co + 127) // 128) for ci, co, _ in LAYERS]

RG = [list(range(NCORES))]
FP8L = {3, 4, 5, 6, 7}   # layers whose matmuls run fp8 DoubleRow
OP8 = {2, 3, 4, 5, 6}    # layers whose output is stored as fp8 pair-tiles

_NC_CACHE = {}


def _build():
    nc = bacc.Bacc(None, target_bir_lowering=False, debug=False)

    x0_d = nc.dram_tensor("x0s", [24, 500], F32, kind="ExternalInput")
    sw_d = nc.dram_tensor("sw", [128, 32], F32, kind="ExternalInput")
    sws_d = nc.dram_tensor("sws24", [24, 500], F16, kind="ExternalInput")
    id_d = nc.dram_tensor("ident8", [8, 8], F16, kind="ExternalInput")
    cneg_d = nc.dram_tensor("cneg", [2, PTS], F16, kind="ExternalInput")
    db3_d = nc.dram_tensor("db3t", [3, NITER], F32, kind="ExternalInput")
    r_d = [[nc.dram_tensor(f"rknn_{i}_{b}", [128, RP], F16,
                           kind="ExternalInput")
            for b in range(BL)] for i in range(NITER)]
    r4_d = [[nc.dram_tensor(f"r4_{i}_{b}", [4, RP], F16, kind="ExternalInput")
             for b in range(BL)] for i in range(NITER)]
    w_d = [[(nc.dram_tensor(f"w_{i}_{l}",
                            [128, NCI[l] // 2, 2,
                             16 if l == 7 else LAYERS[l][1]], F8,
                            kind="ExternalInput") if l in FP8L else
             nc.dram_tensor(f"w_{i}_{l}", list(LAYERS[l][:2]), F16,
                            kind="ExternalInput")) for l in range(8)]
           for i in range(NITER)]
    gb_d = [[nc.dram_tensor(f"gb_{i}_{l}", [128, 2, NCO[l]], F32,
                            kind="ExternalInput") for l in range(7)]
            for i in range(NITER)]
    lossT_d = nc.dram_tensor("lossT", [4, NITER * BL], F32,
                             kind="ExternalOutput")
    loss24_d = nc.dram_tensor("loss24", [24, NITER], F32,
                              kind="ExternalOutput")

    with tile.TileContext(nc) as tc:
        with (
            tc.tile_pool(name="sb", bufs=1) as sb,
            tc.tile_pool(name="ps", bufs=1, space="PSUM") as ps,
            tc.tile_pool(name="dram", bufs=2, space="DRAM") as dram,
        ):
            # ---------- persistent setup ----------
            sw_sb = sb.tile([128, 32], F32, tag="sw")
            nc.sync.dma_start(sw_sb[:], sw_d[:])
            sws24 = sb.tile([24, 500], F16, tag="sws24")
            nc.sync.dma_start(sws24[:], sws_d[:])
            ident = sb.tile([8, 8], F16, tag="ident")
            nc.sync.dma_start(ident[:], id_d[:])
            db3_sb = sb.tile([3, NITER], F32, tag="db3")
            nc.sync.dma_start(db3_sb[:], db3_d[:])
            eps_sb = sb.tile([128, 1], F32, tag="epsc")
            nc.vector.memset(eps_sb[:], float(EPS))

            x24 = sb.tile([24, 500], F32, tag="x24")
            nc.sync.dma_start(x24[:], x0_d[:])
            xf16 = sb.tile([3, NPT, PT], F16, tag="xf")

            # Ld tiles (2, by iter parity), rows replicated at strips 0/32
            Ld2 = [sb.tile([128, PTS], F16, tag=f"Ld{p}", name=f"Ld_{p}")
                   for p in range(2)]
            for p in range(2):
                nc.sync.dma_start(Ld2[p][9:11, :], cneg_d[:])
            m_all2 = [sb.tile([128, 32], F32, tag=f"ma{p}", name=f"ma_{p}")
                      for p in range(2)]
            u4 = sb.tile([128, 32, 4], F16, tag="u4")
            nc.vector.memset(u4[:, :, 3:4], 1.0)

            lossT = sb.tile([4, NITER * BL], F32, tag="lossT")
            nc.vector.memset(lossT[:], 0.0)
            loss24 = sb.tile([24, NITER], F32, tag="loss24")
            nc.vector.memset(loss24[:], 0.0)

            def load_r(it):
                rs, r4s = [], []
                for b in range(BL):
                    rt_ = sb.tile([128, RP], F16, tag=f"r{b}", bufs=2,
                                  name=f"r_{it}_{b}")
                    nc.sync.dma_start(rt_[:], r_d[it][b][:])
                    r4t = sb.tile([4, RP], F16, tag=f"r4_{b}", bufs=2,
                                  name=f"r4t_{it}_{b}")
                    nc.sync.dma_start(r4t[:], r4_d[it][b][:])
                    rs.append(rt_)
                    r4s.append(r4t)
                return rs, r4s

            def refresh_xf16():
                for p in range(NPT):
                    nc.gpsimd.dma_start(xf16[:, p, :],
                                        x24[3 * p:3 * p + 3, :])

            def build_ld(it):
                Ld = Ld2[it % 2]
                qh = sb.tile([24, 500], F16, tag="qh24", bufs=2,
                             name=f"qh_{it}")
                ql = sb.tile([24, 500], F16, tag="ql24", bufs=2,
                             name=f"ql_{it}")
                nc.vector.tensor_scalar_mul(qh[:], x24[:], 2.0)
                nc.vector.scalar_tensor_tensor(ql[:], x24[:], 2.0, qh[:],
                                               OP.mult, OP.subtract)
                for p in range(8):
                    sl = slice(p * 500, (p + 1) * 500)
                    nc.sync.dma_start(Ld[0:3, sl], qh[3 * p:3 * p + 3, :])
                    nc.sync.dma_start(Ld[3:6, sl], ql[3 * p:3 * p + 3, :])
                    nc.sync.dma_start(Ld[6:9, sl], qh[3 * p:3 * p + 3, :])
                # replicate rows 0:11 to strip at partition 32
                nc.sync.dma_start(Ld[32:43, :], Ld[0:11, :])

            # ---- phase 1: d-matmul (2-way row-packed) + row max ----
            def phase1_units(it, r_sb):
                Ld, m_all = Ld2[it % 2], m_all2[it % 2]
                for b in range(BL):
                    for qt in range(8):
                        col = b * 8 + qt
                        qsl = slice(b * N + qt * QT, b * N + (qt + 1) * QT)
                        kpd = ps.tile([QT, 2, 512], F32, tag="kpd", bufs=1,
                                      name=f"kpd1_{it}_{col}")
                        for rt in range(2):
                            st = 32 * rt
                            nc.tensor.matmul(
                                kpd[:, rt, :], Ld[st:st + 11, qsl],
                                r_sb[b][st:st + 11,
                                        rt * 512:(rt + 1) * 512],
                                start=True, stop=True,
                                tile_position=(st, 0))
                        nc.vector.tensor_reduce(m_all[0:QT, col:col + 1],
                                                kpd[:], AX.XY, OP.max)
                        yield

            # ---------- main loop ----------
            r_cur = load_r(0)
            refresh_xf16()
            build_ld(0)

            prevC = None      # generator for phase C of previous iter

            def make_phaseC(it, r_sb, r4_sb):
                Ld, m_all = Ld2[it % 2], m_all2[it % 2]

                def gen():
                    # u4 build: 8 transposes + 1 ts per batch
                    for b in range(BL):
                        xT = ps.tile([QT, 8, 4], F16, tag="Tb", bufs=1,
                                     name=f"xT_{it}_{b}")
                        for qt in range(8):
                            qs = b * N + qt * QT
                            nc.tensor.transpose(
                                xT[:, qt, 0:3],
                                xf16[0:3, (qs // PT), (qs % PT):(qs % PT) + QT],
                                ident[0:3, 0:3])
                        nc.vector.tensor_scalar_mul(
                            u4[0:QT, 8 * b:8 * b + 8, 0:3],
                            xT[:, :, 0:3], -2.0)
                        yield
                    # per (b,qt): recompute d', one-hot*sw, contract
                    for b in range(BL):
                        Tb = ps.tile([4, RP], F32, tag="Tb", bufs=1,
                                     name=f"Tb_{it}_{b}")
                        for qt in range(8):
                            col = b * 8 + qt
                            qsl = slice(b * N + qt * QT,
                                        b * N + (qt + 1) * QT)
                            kpd = ps.tile([QT, 2, 512], F32, tag="kpd",
                                          bufs=1, name=f"kpdC_{it}_{col}")
                            for rt in range(2):
                                st = 32 * rt
                                nc.tensor.matmul(
                                    kpd[:, rt, :], Ld[st:st + 11, qsl],
                                    r_sb[b][st:st + 11,
                                            rt * 512:(rt + 1) * 512],
                                    start=True, stop=True,
                                    tile_position=(st, 0))
                            oh = sb.tile([QT, 2, 512], F16, tag="oh", bufs=2,
                                         name=f"oh_{it}_{col}")
                            nc.vector.tensor_scalar(
                                oh[:], kpd[:], m_all[0:QT, col:col + 1],
                                sw_sb[0:QT, col:col + 1],
                                OP.is_equal, OP.mult)
                            for rt in range(2):
                                nc.tensor.matmul(
                                    Tb[:, rt * 512:(rt + 1) * 512],
                                    u4[0:QT, col, :], oh[:, rt, :],
                                    start=(qt == 0), stop=(qt == 7))
                            yield
                        jk4 = sb.tile([4, RP], F16, tag="jk4", bufs=2,
                                      name=f"jk4_{it}_{b}")
                        nc.vector.scalar_tensor_tensor(
                            jk4[:], Tb[:], 1.0, r4_sb[b][:],
                            OP.mult, OP.mult,
                            accum_out=lossT[0:4, BL * it + b:BL * it + b + 1])
                        yield
                    # term1: sum sw*|x_new|^2
                    wv = sb.tile([24, 500], F16, tag="wv", bufs=2,
                                 name=f"wv_{it}")
                    nc.vector.tensor_tensor(out=wv[:], in0=x24[:],
                                            in1=sws24[:], op=OP.mult)
                    jkw = sb.tile([24, 500], F16, tag="jkw", bufs=2,
                                  name=f"jkw_{it}")
                    nc.vector.scalar_tensor_tensor(
                        jkw[:], wv[:], 1.0, wv[:], OP.mult, OP.mult,
                        accum_out=loss24[0:24, it:it + 1])
                    yield
                return gen()

            def drain(gen, n):
                if gen is None:
                    return
                for _ in range(n):
                    try:
                        next(gen)
                    except StopIteration:
                        break

            for it in range(NITER):
                Ld, m_all = Ld2[it % 2], m_all2[it % 2]
                r_sb, r4_sb = r_cur
                if it + 1 < NITER:
                    r_nxt = load_r(it + 1)

                p1 = phase1_units(it, r_sb)

                # ---------- MLP ----------
                rhs, rhs8 = None, None
                preds3 = sb.tile([3, NPT, PT], F16, tag="preds3",
                                 name=f"preds3_{it}")
                preds24 = sb.tile([24, 500], F32, tag="preds",
                                  name=f"preds_{it}")

                for l in range(8):
                    cin, cout, has_bn = LAYERS[l]
                    nci, nco = NCI[l], NCO[l]
                    CIP = min(128, cin)

                    if has_bn:
                        gb = sb.tile([128, 2, nco], F32, tag="gb", bufs=2,
                                     name=f"gb_{it}_{l}")
                        nc.sync.dma_start(gb[:], gb_d[it][l][:])
                        sums = sb.tile([128, nco, NPT], F32, tag="sums",
                                       bufs=2, name=f"sums_{it}_{l}")
                        statsr = sb.tile([128, 2, nco], F32, tag="statsr",
                                         bufs=2, name=f"statsr_{it}_{l}")
                        if l in OP8:
                            zt8 = [sb.tile([128, 2, PTS], F8,
                                           tag=f"y{l % 2}_{p}",
                                           name=f"y_{it}_{l}_{p}")
                                   for p in range(nco // 2)]
                            zt = None
                        else:
                            zt = [sb.tile([128, PTS], F16,
                                          tag=f"z{l % 2}_{co}",
                                          name=f"z_{it}_{l}_{co}")
                                  for co in range(nco)]
                            zt8 = None

                    def zsl(co, CO, colsl):
                        if zt8 is not None:
                            return zt8[co // 2][0:CO, co % 2, colsl]
                        return zt[co][0:CO, colsl]

                    for co in range(nco):
                        CO = min(128, cout - co * 128)
                        COP = 16 if l == 7 else CO
                        wr = []
                        if l in FP8L:
                            for p in range(nci // 2):
                                w = sb.tile([128, 2, COP], F8,
                                            tag=f"wc{co % 2}_{p}", bufs=2,
                                            name=f"w_{it}_{l}_{co}_{p}")
                                nc.sync.dma_start(
                                    w[:], w_d[it][l][:, p, :,
                                                     co * 128:co * 128 + COP])
                                wr.append(w)
                        else:
                            for ci in range(nci):
                                w = sb.tile([CIP, CO], F16,
                                            tag=f"wc{co % 2}_{ci}", bufs=2,
                                            name=f"w_{it}_{l}_{co}_{ci}")
                                nc.sync.dma_start(
                                    w[:],
                                    w_d[it][l][ci * 128:ci * 128 + CIP,
                                               co * 128:co * 128 + CO])
                                wr.append(w)
                        for pt in range(NPT):
                            ptsl = slice(pt * PT, (pt + 1) * PT)
                            zp = ps.tile([128, PT], F32, tag="zp", bufs=4,
                                         name=f"zp_{it}_{l}_{co}_{pt}")
                            if l in FP8L:
                                for p in range(nci // 2):
                                    nc.tensor.matmul(
                                        zp[0:COP, :], wr[p][:],
                                        rhs8[p][:, :, ptsl],
                                        start=(p == 0),
                                        stop=(p == nci // 2 - 1),
                                        perf_mode=PM.DoubleRow)
                            else:
                                for ci in range(nci):
                                    if l == 0:
                                        rv = xf16[:, pt, :]
                                    else:
                                        rv = rhs[ci][0:CIP, ptsl]
                                    nc.tensor.matmul(
                                        zp[0:CO, :], wr[ci][:], rv,
                                        start=(ci == 0),
                                        stop=(ci == nci - 1))
                            if has_bn:
                                if (co + pt) % 2 == 0:
                                    nc.scalar.activation(
                                        zsl(co, CO, ptsl), zp[0:CO, :],
                                        AF.Copy,
                                        accum_out=sums[0:CO, co, pt:pt + 1])
                                else:
                                    nc.vector.tensor_scalar(
                                        zsl(co, CO, ptsl), zp[0:CO, :],
                                        1.0, 0.0, OP.mult, OP.add,
                                        accum_out=sums[0:CO, co, pt:pt + 1])
                            else:
                                nc.scalar.activation(
                                    preds3[:, pt, :], zp[0:3, :], AF.Tanh,
                                    bias=db3_sb[:, it:it + 1])
                        if has_bn:
                            # sumsq: even chunks on ACT (Square), odd on DVE
                            jk = sb.tile([128, PTS], F16, tag="jksq", bufs=1,
                                         name=f"jksq_{it}_{l}_{co}")
                            if co < nco - 1:
                                if co % 2 == 0:
                                    nc.scalar.activation(
                                        jk[0:CO, :],
                                        zsl(co, CO, slice(0, PTS)),
                                        AF.Square,
                                        accum_out=statsr[0:CO, 1, co:co + 1])
                                else:
                                    nc.vector.scalar_tensor_tensor(
                                        jk[0:CO, :],
                                        zsl(co, CO, slice(0, PTS)), 1.0,
                                        zsl(co, CO, slice(0, PTS)),
                                        OP.mult, OP.mult,
                                        accum_out=statsr[0:CO, 1, co:co + 1])
                            else:
                                sq2 = sb.tile([128, 2], F32, tag="sq2",
                                              bufs=2, name=f"sq2_{it}_{l}")
                                H = PTS // 2
                                for hh in range(2):
                                    eng = nc.scalar if hh == 0 else nc.vector
                                    if hh == 0:
                                        nc.scalar.activation(
                                            jk[0:CO, 0:H],
                                            zsl(co, CO, slice(0, H)),
                                            AF.Square,
                                            accum_out=sq2[0:CO, 0:1])
                                    else:
                                        nc.vector.scalar_tensor_tensor(
                                            jk[0:CO, H:PTS],
                                            zsl(co, CO, slice(H, PTS)), 1.0,
                                            zsl(co, CO, slice(H, PTS)),
                                            OP.mult, OP.mult,
                                            accum_out=sq2[0:CO, 1:2])
                                nc.vector.tensor_reduce(
                                    statsr[0:CO, 1, co:co + 1],
                                    sq2[0:CO, :], AX.X, OP.add)

                    if not has_bn:
                        break

                    nc.vector.tensor_reduce(statsr[:, 0, :], sums[:], AX.X,
                                            OP.add)
                    arin = dram.tile([128, 2, nco], F32, tag="arin")
                    arout = dram.tile([128, 2, nco], F32, tag="arout")
                    nc.sync.dma_start(arin[:], statsr[:])
                    nc.gpsimd.collective_compute(
                        "AllReduce", OP.add, replica_groups=RG,
                        ins=[arin.opt()], outs=[arout.opt()])
                    statsg = sb.tile([128, 2, nco], F32, tag="statsg", bufs=2,
                                     name=f"statsg_{it}_{l}")
                    nc.sync.dma_start(statsg[:], arout[:])

                    # --- filler work into the AR gap (tensor + DVE) ---
                    if l <= 3:
                        drain(prevC, 15)
                    else:
                        drain(prevC, 99)
                        drain(p1, 12 if l in (4, 5) else 99)

                    # --- af chain ---
                    af = sb.tile([128, 6, nco], F32, tag="af", bufs=2,
                                 name=f"af_{it}_{l}")
                    inv_n = 1.0 / NPTS_GLOBAL
                    nc.vector.tensor_scalar_mul(af[:, 2, :], statsg[:, 0, :],
                                                inv_n)  # m
                    nc.vector.scalar_tensor_tensor(
                        af[:, 3, :], statsg[:, 0, :], inv_n,
                        af[:, 2, :], OP.mult, OP.mult)  # m^2
                    nc.vector.scalar_tensor_tensor(
                        af[:, 3, :], statsg[:, 1, :], inv_n,
                        af[:, 3, :], OP.mult, OP.subtract)  # v
                    nc.scalar.activation(af[:, 4, :], af[:, 3, :], AF.Sqrt,
                                         bias=eps_sb[:])
                    nc.vector.reciprocal(af[:, 5, :], af[:, 4, :])
                    nc.vector.tensor_tensor(out=af[:, 0, :], in0=gb[:, 0, :],
                                            in1=af[:, 5, :], op=OP.mult)
                    nc.vector.tensor_tensor(out=af[:, 4, :], in0=af[:, 2, :],
                                            in1=af[:, 0, :], op=OP.mult)
                    nc.vector.tensor_tensor(out=af[:, 1, :], in0=gb[:, 1, :],
                                            in1=af[:, 4, :], op=OP.subtract)

                    # --- affine+relu: per pt tile, alternating engines ---
                    def _aff(co, CO, col, eng):
                        dst = zsl(co, CO, col)
                        if eng == 0:
                            nc.scalar.activation(
                                dst, dst, AF.Relu,
                                bias=af[0:CO, 1, co:co + 1],
                                scale=af[0:CO, 0, co:co + 1])
                        else:
                            nc.vector.tensor_scalar(
                                dst, dst,
                                af[0:CO, 0, co:co + 1],
                                af[0:CO, 1, co:co + 1], OP.mult, OP.add)
                            nc.vector.tensor_scalar_max(dst, dst, 0.0)
                    for sl4 in range(2):
                        for co in range(nco):
                            CO = min(128, cout - co * 128)
                            _aff(co, CO, slice(sl4 * 2000, (sl4 + 1) * 2000),
                                 (co + sl4) % 2)
                    rhs, rhs8 = zt, zt8

                # ---------- x update; spill leftover filler ----------
                drain(prevC, 99)
                drain(p1, 99)
                for p in range(NPT):
                    nc.gpsimd.dma_start(preds24[3 * p:3 * p + 3, :],
                                        preds3[:, p, :])
                nc.vector.tensor_tensor(out=x24[:], in0=x24[:],
                                        in1=preds24[:], op=OP.add)
                refresh_xf16()
                if it + 1 < NITER:
                    build_ld(it + 1)
                prevC = make_phaseC(it, r_sb, r4_sb)
                if it + 1 < NITER:
                    r_cur = r_nxt

            # tail: phase C of last iter
            drain(prevC, 999)

            nc.sync.dma_start(lossT_d[:], lossT[:])
            nc.sync.dma_start(loss24_d[:], loss24[:])
    nc.compile()
    return nc


def _host_prep(inputs):
    f32 = np.float32
    noisy = np.asarray(inputs["pcl_noisy"], f32)
    clean = np.asarray(inputs["pcl_clean"], f32)
    seeds = np.asarray(inputs["pcl_seeds"], f32)
    std = np.asarray(inputs["pcl_std"], f32)
    noise = np.asarray(inputs["noise"], f32)

    pn = noisy - seeds
    pc = clean - seeds
    sdist = np.sum(pn.astype(np.float64) ** 2, -1, keepdims=True)
    max_sq = sdist[:, -1:, :]
    sw = np.exp(-sdist * 9.0 / max_sq)[..., 0]
    sw = (sw / sw.sum(1, keepdims=True))  # [B, N] float64

    tgts = []
    cur = std.copy()
    for i in range(NITER):
        if i < NITER - 1:
            cur = cur / NOISE_DECAY
            tgts.append(pc + noise[i] * cur[:, None, None])
        else:
            tgts.append(pc.copy())

    sent = np.full((RP - N, 3), 100.0, np.float64)

    shared = {}
    f8np = mybir.dt.np(F8)
    for i in range(NITER):
        for l in range(8):
            key = f'ew{l+1}' if l < 5 else f'dw{l-4}'
            W = np.asarray(inputs[key], f32)[i]
            if l in FP8L:
                P2 = NCI[l] // 2
                cw = LAYERS[l][1]
                if l == 7:
                    Wp = np.zeros((W.shape[0], 16), f32)
                    Wp[:, :cw] = W
                    W, cw = Wp, 16
                arr = W.reshape(P2, 2, 128, cw).transpose(2, 0, 1, 3)
                shared[f"w_{i}_{l}"] = np.ascontiguousarray(arr).astype(f8np)
            else:
                shared[f"w_{i}_{l}"] = W.astype(np.float16)
        for l in range(7):
            nco = NCO[l]
            cout = LAYERS[l][1]
            gk = f'eg{l+1}' if l < 5 else f'dg{l-4}'
            hk = f'eh{l+1}' if l < 5 else f'dh{l-4}'
            g = np.asarray(inputs[gk], f32)[i]
            h = np.asarray(inputs[hk], f32)[i]
            arr = np.zeros((128, 2, nco), f32)
            gp = np.zeros(nco * 128, f32); gp[:cout] = g
            hp = np.zeros(nco * 128, f32); hp[:cout] = h
            arr[:, 0, :] = gp.reshape(nco, 128).T
            arr[:, 1, :] = hp.reshape(nco, 128).T
            shared[f"gb_{i}_{l}"] = arr
    shared["db3t"] = np.ascontiguousarray(np.asarray(inputs["db3"], f32).T)
    shared["ident8"] = np.eye(8, dtype=np.float16)
    shared["cneg"] = np.full((2, PTS), -1.0, np.float16)

    in_maps = []
    for c in range(NCORES):
        bs = slice(c * BL, (c + 1) * BL)
        m = dict(shared)
        x3 = pn[bs].transpose(2, 0, 1).reshape(3, PTS)  # [3, 4000]
        m["x0s"] = np.ascontiguousarray(
            x3.reshape(3, 8, 500).transpose(1, 0, 2).reshape(24, 500))
        swc = np.zeros((128, 32), f32)
        for b in range(BL):
            for qt in range(8):
                swc[0:QT, b * 8 + qt] = sw[c * BL + b,
                                           qt * QT:(qt + 1) * QT].astype(f32)
        m["sw"] = swc
        sq = np.sqrt(sw[bs]).reshape(1, PTS)
        sq3 = np.broadcast_to(sq, (3, PTS)).astype(np.float16)
        m["sws24"] = np.ascontiguousarray(
            sq3.reshape(3, 8, 500).transpose(1, 0, 2).reshape(24, 500))
        for i in range(NITER):
            for b in range(BL):
                coords = np.concatenate(
                    [tgts[i][c * BL + b].astype(np.float64), sent], 0)
                rh = coords.astype(np.float16)
                rl = (coords - rh.astype(np.float64)).astype(np.float16)
                rsq = (coords ** 2).sum(1)
                rsqh = rsq.astype(np.float16)
                rsql = (rsq - rsqh.astype(np.float64)).astype(np.float16)
                R = np.zeros((128, RP), np.float16)
                for st in (0, 32):
                    R[st + 0:st + 3] = rh.T
                    R[st + 3:st + 6] = rh.T
                    R[st + 6:st + 9] = rl.T
                    R[st + 9] = rsqh
                    R[st + 10] = rsql
                m[f"rknn_{i}_{b}"] = R
                R4 = np.empty((4, RP), np.float16)
                R4[0:3] = coords.T
                R4[3] = rsq
                m[f"r4_{i}_{b}"] = R4
        in_maps.append(m)
    return in_maps


def kernel(**inputs):
    if "nc" not in _NC_CACHE:
        _NC_CACHE["nc"] = _build()
    nc = _NC_CACHE["nc"]
    in_maps = _host_prep(inputs)
    res = run_bass_kernel_spmd(nc, in_maps, list(range(NCORES))).results
    total = 0.0
    for c in range(NCORES):
        total += float(res[c]["lossT"].sum())
        total += float(res[c]["loss24"].sum())
    return np.asarray(total / B, dtype=np.float32)


# revision 35
# speedup vs baseline: 1.0525x; 1.0525x over previous
"""DenoiseNet loss kernel for 8 Trainium2 NeuronCores.

Data parallel over batch (4/core). Exact global BatchNorm via per-layer
AllReduce of (sum, sumsq); BN+ReLU applied as one affine relu(a*h+c).
Big MLP layers run fp8 DoubleRow (2 contract rows/cycle). KNN d'=2x.r-|r|^2
via fp16 hi/lo matmuls row-packed 2-wide with tile_position; argmax by
reduce-max; loss extracted by a one-hot contraction matmul
T=sum_q onehot*sw*[-2x_new^T;1] then sum_r T*[r;|r|^2]. KNN work fills
the AllReduce latency gaps.
"""
import numpy as np

import concourse.bass as bass
import concourse.mybir as mybir
import concourse.tile as tile
from concourse import bacc
from concourse.bass_utils import run_bass_kernel_spmd

dt = mybir.dt
F32 = dt.float32
F16 = dt.float16
F8 = dt.float8e4
AF = mybir.ActivationFunctionType
OP = mybir.AluOpType
AX = mybir.AxisListType
PM = mybir.MatmulPerfMode

B, N, NCORES = 32, 1000, 8
BL = B // NCORES            # 4 batches per core
PTS = BL * N                # 4000 points per core
NITER = 4
NPTS_GLOBAL = B * N         # 32000 (BN population)
EPS = 1e-5
NOISE_DECAY = 4.0
QT = 125                    # q tile (8 per batch)
RP = 1024                   # padded ref points (24 sentinels)
PT = 500                    # pts tile for MLP
NPT = PTS // PT             # 8

LAYERS = [(3, 64, 1), (64, 128, 1), (128, 256, 1), (256, 512, 1),
          (512, 1024, 1), (1024, 512, 1), (512, 256, 1), (256, 3, 0)]
NCI = [max(1, ci // 128) for ci, co, _ in LAYERS]
NCO = [max(1, (co + 127) // 128) for ci, co, _ in LAYERS]

RG = [list(range(NCORES))]
FP8L = {3, 4, 5, 6, 7}   # layers whose matmuls run fp8 DoubleRow
SPLIT_AR = {4, 5}        # layers whose stats AllReduce is split in halves
OP8 = {2, 3, 4, 5, 6}    # layers whose output is stored as fp8 pair-tiles

_NC_CACHE = {}


def _build():
    nc = bacc.Bacc(None, target_bir_lowering=False, debug=False)

    x0_d = nc.dram_tensor("x0s", [24, 500], F32, kind="ExternalInput")
    sw_d = nc.dram_tensor("sw", [128, 32], F32, kind="ExternalInput")
    sws_d = nc.dram_tensor("sws24", [24, 500], F16, kind="ExternalInput")
    id_d = nc.dram_tensor("ident8", [8, 8], F16, kind="ExternalInput")
    cneg_d = nc.dram_tensor("cneg", [2, PTS], F16, kind="ExternalInput")
    db3_d = nc.dram_tensor("db3t", [3, NITER], F32, kind="ExternalInput")
    r_d = [[nc.dram_tensor(f"rknn_{i}_{b}", [128, RP], F16,
                           kind="ExternalInput")
            for b in range(BL)] for i in range(NITER)]
    r4_d = [[nc.dram_tensor(f"r4_{i}_{b}", [4, RP], F16, kind="ExternalInput")
             for b in range(BL)] for i in range(NITER)]
    w_d = [[(nc.dram_tensor(f"w_{i}_{l}",
                            [128, NCI[l] // 2, 2,
                             16 if l == 7 else LAYERS[l][1]], F8,
                            kind="ExternalInput") if l in FP8L else
             nc.dram_tensor(f"w_{i}_{l}", list(LAYERS[l][:2]), F16,
                            kind="ExternalInput")) for l in range(8)]
           for i in range(NITER)]
    gb_d = [[nc.dram_tensor(f"gb_{i}_{l}", [128, 2, NCO[l]], F32,
                            kind="ExternalInput") for l in range(7)]
            for i in range(NITER)]
    lossT_d = nc.dram_tensor("lossT", [4, NITER * BL], F32,
                             kind="ExternalOutput")
    loss24_d = nc.dram_tensor("loss24", [24, NITER], F32,
                              kind="ExternalOutput")

    with tile.TileContext(nc) as tc:
        with (
            tc.tile_pool(name="sb", bufs=1) as sb,
            tc.tile_pool(name="ps", bufs=1, space="PSUM") as ps,
            tc.tile_pool(name="dram", bufs=2, space="DRAM") as dram,
        ):
            # ---------- persistent setup ----------
            sw_sb = sb.tile([128, 32], F32, tag="sw")
            nc.sync.dma_start(sw_sb[:], sw_d[:])
            sws24 = sb.tile([24, 500], F16, tag="sws24")
            nc.sync.dma_start(sws24[:], sws_d[:])
            ident = sb.tile([8, 8], F16, tag="ident")
            nc.sync.dma_start(ident[:], id_d[:])
            db3_sb = sb.tile([3, NITER], F32, tag="db3")
            nc.sync.dma_start(db3_sb[:], db3_d[:])
            eps_sb = sb.tile([128, 1], F32, tag="epsc")
            nc.vector.memset(eps_sb[:], float(EPS))

            x24 = sb.tile([24, 500], F32, tag="x24")
            nc.sync.dma_start(x24[:], x0_d[:])
            xf16 = sb.tile([3, NPT, PT], F16, tag="xf")

            # Ld tiles (2, by iter parity), rows replicated at strips 0/32
            Ld2 = [sb.tile([128, PTS], F16, tag=f"Ld{p}", name=f"Ld_{p}")
                   for p in range(2)]
            for p in range(2):
                nc.sync.dma_start(Ld2[p][9:11, :], cneg_d[:])
            m_all2 = [sb.tile([128, 32], F32, tag=f"ma{p}", name=f"ma_{p}")
                      for p in range(2)]
            u4 = sb.tile([128, 32, 4], F16, tag="u4")
            nc.vector.memset(u4[:, :, 3:4], 1.0)

            lossT = sb.tile([4, NITER * BL], F32, tag="lossT")
            nc.vector.memset(lossT[:], 0.0)
            loss24 = sb.tile([24, NITER], F32, tag="loss24")
            nc.vector.memset(loss24[:], 0.0)

            def load_r(it):
                rs, r4s = [], []
                for b in range(BL):
                    rt_ = sb.tile([128, RP], F16, tag=f"r{b}", bufs=2,
                                  name=f"r_{it}_{b}")
                    nc.sync.dma_start(rt_[:], r_d[it][b][:])
                    r4t = sb.tile([4, RP], F16, tag=f"r4_{b}", bufs=2,
                                  name=f"r4t_{it}_{b}")
                    nc.sync.dma_start(r4t[:], r4_d[it][b][:])
                    rs.append(rt_)
                    r4s.append(r4t)
                return rs, r4s

            def refresh_xf16():
                for p in range(NPT):
                    nc.gpsimd.dma_start(xf16[:, p, :],
                                        x24[3 * p:3 * p + 3, :])

            def build_ld(it):
                Ld = Ld2[it % 2]
                qh = sb.tile([24, 500], F16, tag="qh24", bufs=2,
                             name=f"qh_{it}")
                ql = sb.tile([24, 500], F16, tag="ql24", bufs=2,
                             name=f"ql_{it}")
                nc.vector.tensor_scalar_mul(qh[:], x24[:], 2.0)
                nc.vector.scalar_tensor_tensor(ql[:], x24[:], 2.0, qh[:],
                                               OP.mult, OP.subtract)
                for p in range(8):
                    sl = slice(p * 500, (p + 1) * 500)
                    nc.sync.dma_start(Ld[0:3, sl], qh[3 * p:3 * p + 3, :])
                    nc.sync.dma_start(Ld[3:6, sl], ql[3 * p:3 * p + 3, :])
                    nc.sync.dma_start(Ld[6:9, sl], qh[3 * p:3 * p + 3, :])
                # replicate rows 0:11 to strip at partition 32
                nc.sync.dma_start(Ld[32:43, :], Ld[0:11, :])

            # ---- phase 1: d-matmul (2-way row-packed) + row max ----
            def phase1_units(it, r_sb):
                Ld, m_all = Ld2[it % 2], m_all2[it % 2]
                for b in range(BL):
                    for qt in range(8):
                        col = b * 8 + qt
                        qsl = slice(b * N + qt * QT, b * N + (qt + 1) * QT)
                        kpd = ps.tile([QT, 2, 512], F32, tag="kpd", bufs=1,
                                      name=f"kpd1_{it}_{col}")
                        for rt in range(2):
                            st = 32 * rt
                            nc.tensor.matmul(
                                kpd[:, rt, :], Ld[st:st + 11, qsl],
                                r_sb[b][st:st + 11,
                                        rt * 512:(rt + 1) * 512],
                                start=True, stop=True,
                                tile_position=(st, 0))
                        nc.vector.tensor_reduce(m_all[0:QT, col:col + 1],
                                                kpd[:], AX.XY, OP.max)
                        yield

            # ---------- main loop ----------
            r_cur = load_r(0)
            refresh_xf16()
            build_ld(0)

            prevC = None      # generator for phase C of previous iter

            def make_phaseC(it, r_sb, r4_sb):
                Ld, m_all = Ld2[it % 2], m_all2[it % 2]

                def gen():
                    # u4 build: 8 transposes + 1 ts per batch
                    for b in range(BL):
                        xT = ps.tile([QT, 8, 4], F16, tag="Tb", bufs=1,
                                     name=f"xT_{it}_{b}")
                        for qt in range(8):
                            qs = b * N + qt * QT
                            nc.tensor.transpose(
                                xT[:, qt, 0:3],
                                xf16[0:3, (qs // PT), (qs % PT):(qs % PT) + QT],
                                ident[0:3, 0:3])
                        nc.vector.tensor_scalar_mul(
                            u4[0:QT, 8 * b:8 * b + 8, 0:3],
                            xT[:, :, 0:3], -2.0)
                        yield
                    # per (b,qt): recompute d', one-hot*sw, contract
                    for b in range(BL):
                        Tb = ps.tile([4, RP], F32, tag="Tb", bufs=1,
                                     name=f"Tb_{it}_{b}")
                        for qt in range(8):
                            col = b * 8 + qt
                            qsl = slice(b * N + qt * QT,
                                        b * N + (qt + 1) * QT)
                            kpd = ps.tile([QT, 2, 512], F32, tag="kpd",
                                          bufs=1, name=f"kpdC_{it}_{col}")
                            for rt in range(2):
                                st = 32 * rt
                                nc.tensor.matmul(
                                    kpd[:, rt, :], Ld[st:st + 11, qsl],
                                    r_sb[b][st:st + 11,
                                            rt * 512:(rt + 1) * 512],
                                    start=True, stop=True,
                                    tile_position=(st, 0))
                            oh = sb.tile([QT, 2, 512], F16, tag="oh", bufs=2,
                                         name=f"oh_{it}_{col}")
                            nc.vector.tensor_scalar(
                                oh[:], kpd[:], m_all[0:QT, col:col + 1],
                                sw_sb[0:QT, col:col + 1],
                                OP.is_equal, OP.mult)
                            for rt in range(2):
                                nc.tensor.matmul(
                                    Tb[:, rt * 512:(rt + 1) * 512],
                                    u4[0:QT, col, :], oh[:, rt, :],
                                    start=(qt == 0), stop=(qt == 7))
                            yield
                        jk4 = sb.tile([4, RP], F16, tag="jk4", bufs=2,
                                      name=f"jk4_{it}_{b}")
                        nc.vector.scalar_tensor_tensor(
                            jk4[:], Tb[:], 1.0, r4_sb[b][:],
                            OP.mult, OP.mult,
                            accum_out=lossT[0:4, BL * it + b:BL * it + b + 1])
                        yield
                    # term1: sum sw*|x_new|^2
                    wv = sb.tile([24, 500], F16, tag="wv", bufs=2,
                                 name=f"wv_{it}")
                    nc.vector.tensor_tensor(out=wv[:], in0=x24[:],
                                            in1=sws24[:], op=OP.mult)
                    jkw = sb.tile([24, 500], F16, tag="jkw", bufs=2,
                                  name=f"jkw_{it}")
                    nc.vector.scalar_tensor_tensor(
                        jkw[:], wv[:], 1.0, wv[:], OP.mult, OP.mult,
                        accum_out=loss24[0:24, it:it + 1])
                    yield
                return gen()

            def drain(gen, n):
                if gen is None:
                    return
                for _ in range(n):
                    try:
                        next(gen)
                    except StopIteration:
                        break

            for it in range(NITER):
                Ld, m_all = Ld2[it % 2], m_all2[it % 2]
                r_sb, r4_sb = r_cur
                if it + 1 < NITER:
                    r_nxt = load_r(it + 1)

                p1 = phase1_units(it, r_sb)

                # ---------- MLP ----------
                rhs, rhs8 = None, None
                preds3 = sb.tile([3, NPT, PT], F16, tag="preds3",
                                 name=f"preds3_{it}")
                preds24 = sb.tile([24, 500], F32, tag="preds",
                                  name=f"preds_{it}")

                for l in range(8):
                    cin, cout, has_bn = LAYERS[l]
                    nci, nco = NCI[l], NCO[l]
                    CIP = min(128, cin)

                    if has_bn:
                        gb = sb.tile([128, 2, nco], F32, tag="gb", bufs=2,
                                     name=f"gb_{it}_{l}")
                        nc.sync.dma_start(gb[:], gb_d[it][l][:])
                        sums = sb.tile([128, nco, NPT], F32, tag="sums",
                                       bufs=2, name=f"sums_{it}_{l}")
                        statsr = sb.tile([128, 2, nco], F32, tag="statsr",
                                         bufs=2, name=f"statsr_{it}_{l}")
                        if l in OP8:
                            zt8 = [sb.tile([128, 2, PTS], F8,
                                           tag=f"y{l % 2}_{p}",
                                           name=f"y_{it}_{l}_{p}")
                                   for p in range(nco // 2)]
                            zt = None
                        else:
                            zt = [sb.tile([128, PTS], F16,
                                          tag=f"z{l % 2}_{co}",
                                          name=f"z_{it}_{l}_{co}")
                                  for co in range(nco)]
                            zt8 = None

                    def zsl(co, CO, colsl):
                        if zt8 is not None:
                            return zt8[co // 2][0:CO, co % 2, colsl]
                        return zt[co][0:CO, colsl]

                    for co in range(nco):
                        CO = min(128, cout - co * 128)
                        COP = 16 if l == 7 else CO
                        wr = []
                        if l in FP8L:
                            for p in range(nci // 2):
                                w = sb.tile([128, 2, COP], F8,
                                            tag=f"wc{co % 2}_{p}", bufs=2,
                                            name=f"w_{it}_{l}_{co}_{p}")
                                nc.sync.dma_start(
                                    w[:], w_d[it][l][:, p, :,
                                                     co * 128:co * 128 + COP])
                                wr.append(w)
                        else:
                            for ci in range(nci):
                                w = sb.tile([CIP, CO], F16,
                                            tag=f"wc{co % 2}_{ci}", bufs=2,
                                            name=f"w_{it}_{l}_{co}_{ci}")
                                nc.sync.dma_start(
                                    w[:],
                                    w_d[it][l][ci * 128:ci * 128 + CIP,
                                               co * 128:co * 128 + CO])
                                wr.append(w)
                        for pt in range(NPT):
                            ptsl = slice(pt * PT, (pt + 1) * PT)
                            zp = ps.tile([128, PT], F32, tag="zp", bufs=4,
                                         name=f"zp_{it}_{l}_{co}_{pt}")
                            if l in FP8L:
                                for p in range(nci // 2):
                                    nc.tensor.matmul(
                                        zp[0:COP, :], wr[p][:],
                                        rhs8[p][:, :, ptsl],
                                        start=(p == 0),
                                        stop=(p == nci // 2 - 1),
                                        perf_mode=PM.DoubleRow)
                            else:
                                for ci in range(nci):
                                    if l == 0:
                                        rv = xf16[:, pt, :]
                                    else:
                                        rv = rhs[ci][0:CIP, ptsl]
                                    nc.tensor.matmul(
                                        zp[0:CO, :], wr[ci][:], rv,
                                        start=(ci == 0),
                                        stop=(ci == nci - 1))
                            if has_bn:
                                nc.scalar.activation(
                                    zsl(co, CO, ptsl), zp[0:CO, :], AF.Copy,
                                    accum_out=sums[0:CO, co, pt:pt + 1])
                            else:
                                nc.scalar.activation(
                                    preds3[:, pt, :], zp[0:3, :], AF.Tanh,
                                    bias=db3_sb[:, it:it + 1])
                        if has_bn:
                            # sumsq: even chunks on ACT (Square), odd on DVE
                            jk = sb.tile([128, PTS], F16, tag="jksq", bufs=1,
                                         name=f"jksq_{it}_{l}_{co}")
                            if co < nco - 1:
                                if co % 2 == 0:
                                    nc.scalar.activation(
                                        jk[0:CO, :],
                                        zsl(co, CO, slice(0, PTS)),
                                        AF.Square,
                                        accum_out=statsr[0:CO, 1, co:co + 1])
                                else:
                                    nc.vector.scalar_tensor_tensor(
                                        jk[0:CO, :],
                                        zsl(co, CO, slice(0, PTS)), 1.0,
                                        zsl(co, CO, slice(0, PTS)),
                                        OP.mult, OP.mult,
                                        accum_out=statsr[0:CO, 1, co:co + 1])
                            else:
                                sq2 = sb.tile([128, 2], F32, tag="sq2",
                                              bufs=2, name=f"sq2_{it}_{l}")
                                H = PTS // 2
                                for hh in range(2):
                                    eng = nc.scalar if hh == 0 else nc.vector
                                    if hh == 0:
                                        nc.scalar.activation(
                                            jk[0:CO, 0:H],
                                            zsl(co, CO, slice(0, H)),
                                            AF.Square,
                                            accum_out=sq2[0:CO, 0:1])
                                    else:
                                        nc.vector.scalar_tensor_tensor(
                                            jk[0:CO, H:PTS],
                                            zsl(co, CO, slice(H, PTS)), 1.0,
                                            zsl(co, CO, slice(H, PTS)),
                                            OP.mult, OP.mult,
                                            accum_out=sq2[0:CO, 1:2])
                                nc.vector.tensor_reduce(
                                    statsr[0:CO, 1, co:co + 1],
                                    sq2[0:CO, :], AX.X, OP.add)
                        if (l in SPLIT_AR and has_bn
                                and co == nco // 2 - 1):
                            h = nco // 2
                            nc.vector.tensor_reduce(
                                statsr[:, 0, 0:h], sums[:, 0:h, :], AX.X,
                                OP.add)
                            arinA = dram.tile([128, 2, h], F32, tag="arin")
                            aroutA = dram.tile([128, 2, h], F32, tag="arout")
                            nc.sync.dma_start(arinA[:], statsr[:, :, 0:h])
                            nc.gpsimd.collective_compute(
                                "AllReduce", OP.add, replica_groups=RG,
                                ins=[arinA.opt()], outs=[aroutA.opt()])
                            statsg_A = sb.tile([128, 2, h], F32,
                                               tag="statsgA", bufs=2,
                                               name=f"statsgA_{it}_{l}")
                            nc.sync.dma_start(statsg_A[:], aroutA[:])

                    if not has_bn:
                        break

                    af = sb.tile([128, 6, nco], F32, tag="af", bufs=2,
                                 name=f"af_{it}_{l}")
                    inv_n = 1.0 / NPTS_GLOBAL

                    def _aff(co, CO, col, eng):
                        dst = zsl(co, CO, col)
                        if eng == 0:
                            nc.scalar.activation(
                                dst, dst, AF.Relu,
                                bias=af[0:CO, 1, co:co + 1],
                                scale=af[0:CO, 0, co:co + 1])
                        else:
                            nc.vector.tensor_scalar(
                                dst, dst,
                                af[0:CO, 0, co:co + 1],
                                af[0:CO, 1, co:co + 1], OP.mult, OP.add)
                            nc.vector.tensor_scalar_max(dst, dst, 0.0)

                    def af_half(sg, h0, h1):
                        hs = slice(h0, h1)
                        nc.vector.tensor_scalar_mul(af[:, 2, hs], sg[:, 0, :],
                                                    inv_n)  # m
                        nc.vector.scalar_tensor_tensor(
                            af[:, 3, hs], sg[:, 0, :], inv_n,
                            af[:, 2, hs], OP.mult, OP.mult)  # m^2
                        nc.vector.scalar_tensor_tensor(
                            af[:, 3, hs], sg[:, 1, :], inv_n,
                            af[:, 3, hs], OP.mult, OP.subtract)  # v
                        nc.scalar.activation(af[:, 4, hs], af[:, 3, hs],
                                             AF.Sqrt, bias=eps_sb[:])
                        nc.vector.reciprocal(af[:, 5, hs], af[:, 4, hs])
                        nc.vector.tensor_tensor(out=af[:, 0, hs],
                                                in0=gb[:, 0, hs],
                                                in1=af[:, 5, hs], op=OP.mult)
                        nc.vector.tensor_tensor(out=af[:, 4, hs],
                                                in0=af[:, 2, hs],
                                                in1=af[:, 0, hs], op=OP.mult)
                        nc.vector.tensor_tensor(out=af[:, 1, hs],
                                                in0=gb[:, 1, hs],
                                                in1=af[:, 4, hs],
                                                op=OP.subtract)

                    if l in SPLIT_AR:
                        h = nco // 2
                        nc.vector.tensor_reduce(statsr[:, 0, h:nco],
                                                sums[:, h:nco, :], AX.X,
                                                OP.add)
                        arin = dram.tile([128, 2, nco - h], F32, tag="arin")
                        arout = dram.tile([128, 2, nco - h], F32, tag="arout")
                        nc.sync.dma_start(arin[:], statsr[:, :, h:nco])
                        nc.gpsimd.collective_compute(
                            "AllReduce", OP.add, replica_groups=RG,
                            ins=[arin.opt()], outs=[arout.opt()])
                        statsg = sb.tile([128, 2, nco - h], F32, tag="statsg",
                                         bufs=2, name=f"statsg_{it}_{l}")
                        nc.sync.dma_start(statsg[:], arout[:])
                        af_half(statsg_A, 0, h)
                        for sl4 in range(4):
                            for co in range(h):
                                CO = min(128, cout - co * 128)
                                _aff(co, CO,
                                     slice(sl4 * 1000, (sl4 + 1) * 1000),
                                     (co + sl4) % 2)
                        drain(prevC, 99)
                        drain(p1, 12)
                        af_half(statsg, h, nco)
                    else:
                        nc.vector.tensor_reduce(statsr[:, 0, :], sums[:],
                                                AX.X, OP.add)
                        arin = dram.tile([128, 2, nco], F32, tag="arin")
                        arout = dram.tile([128, 2, nco], F32, tag="arout")
                        nc.sync.dma_start(arin[:], statsr[:])
                        nc.gpsimd.collective_compute(
                            "AllReduce", OP.add, replica_groups=RG,
                            ins=[arin.opt()], outs=[arout.opt()])
                        statsg = sb.tile([128, 2, nco], F32, tag="statsg",
                                         bufs=2, name=f"statsg_{it}_{l}")
                        nc.sync.dma_start(statsg[:], arout[:])
                        if l <= 3:
                            drain(prevC, 15)
                        else:
                            drain(prevC, 99)
                            drain(p1, 99)
                        af_half(statsg, 0, nco)

                    # --- affine+relu: per pt tile, alternating engines ---
                    co_lo = nco // 2 if l in SPLIT_AR else 0
                    for sl4 in range(4):
                        for co in range(co_lo, nco):
                            CO = min(128, cout - co * 128)
                            _aff(co, CO, slice(sl4 * 1000, (sl4 + 1) * 1000),
                                 (co + sl4) % 2)
                    rhs, rhs8 = zt, zt8

                # ---------- x update; spill leftover filler ----------
                drain(prevC, 99)
                drain(p1, 99)
                for p in range(NPT):
                    nc.gpsimd.dma_start(preds24[3 * p:3 * p + 3, :],
                                        preds3[:, p, :])
                nc.vector.tensor_tensor(out=x24[:], in0=x24[:],
                                        in1=preds24[:], op=OP.add)
                refresh_xf16()
                if it + 1 < NITER:
                    build_ld(it + 1)
                prevC = make_phaseC(it, r_sb, r4_sb)
                if it + 1 < NITER:
                    r_cur = r_nxt

            # tail: phase C of last iter
            drain(prevC, 999)

            nc.sync.dma_start(lossT_d[:], lossT[:])
            nc.sync.dma_start(loss24_d[:], loss24[:])
    nc.compile()
    return nc


def _host_prep(inputs):
    f32 = np.float32
    noisy = np.asarray(inputs["pcl_noisy"], f32)
    clean = np.asarray(inputs["pcl_clean"], f32)
    seeds = np.asarray(inputs["pcl_seeds"], f32)
    std = np.asarray(inputs["pcl_std"], f32)
    noise = np.asarray(inputs["noise"], f32)

    pn = noisy - seeds
    pc = clean - seeds
    sdist = np.sum(pn.astype(np.float64) ** 2, -1, keepdims=True)
    max_sq = sdist[:, -1:, :]
    sw = np.exp(-sdist * 9.0 / max_sq)[..., 0]
    sw = (sw / sw.sum(1, keepdims=True))  # [B, N] float64

    tgts = []
    cur = std.copy()
    for i in range(NITER):
        if i < NITER - 1:
            cur = cur / NOISE_DECAY
            tgts.append(pc + noise[i] * cur[:, None, None])
        else:
            tgts.append(pc.copy())

    sent = np.full((RP - N, 3), 100.0, np.float64)

    shared = {}
    f8np = mybir.dt.np(F8)
    for i in range(NITER):
        for l in range(8):
            key = f'ew{l+1}' if l < 5 else f'dw{l-4}'
            W = np.asarray(inputs[key], f32)[i]
            if l in FP8L:
                P2 = NCI[l] // 2
                cw = LAYERS[l][1]
                if l == 7:
                    Wp = np.zeros((W.shape[0], 16), f32)
                    Wp[:, :cw] = W
                    W, cw = Wp, 16
                arr = W.reshape(P2, 2, 128, cw).transpose(2, 0, 1, 3)
                shared[f"w_{i}_{l}"] = np.ascontiguousarray(arr).astype(f8np)
            else:
                shared[f"w_{i}_{l}"] = W.astype(np.float16)
        for l in range(7):
            nco = NCO[l]
            cout = LAYERS[l][1]
            gk = f'eg{l+1}' if l < 5 else f'dg{l-4}'
            hk = f'eh{l+1}' if l < 5 else f'dh{l-4}'
            g = np.asarray(inputs[gk], f32)[i]
            h = np.asarray(inputs[hk], f32)[i]
            arr = np.zeros((128, 2, nco), f32)
            gp = np.zeros(nco * 128, f32); gp[:cout] = g
            hp = np.zeros(nco * 128, f32); hp[:cout] = h
            arr[:, 0, :] = gp.reshape(nco, 128).T
            arr[:, 1, :] = hp.reshape(nco, 128).T
            shared[f"gb_{i}_{l}"] = arr
    shared["db3t"] = np.ascontiguousarray(np.asarray(inputs["db3"], f32).T)
    shared["ident8"] = np.eye(8, dtype=np.float16)
    shared["cneg"] = np.full((2, PTS), -1.0, np.float16)

    in_maps = []
    for c in range(NCORES):
        bs = slice(c * BL, (c + 1) * BL)
        m = dict(shared)
        x3 = pn[bs].transpose(2, 0, 1).reshape(3, PTS)  # [3, 4000]
        m["x0s"] = np.ascontiguousarray(
            x3.reshape(3, 8, 500).transpose(1, 0, 2).reshape(24, 500))
        swc = np.zeros((128, 32), f32)
        for b in range(BL):
            for qt in range(8):
                swc[0:QT, b * 8 + qt] = sw[c * BL + b,
                                           qt * QT:(qt + 1) * QT].astype(f32)
        m["sw"] = swc
        sq = np.sqrt(sw[bs]).reshape(1, PTS)
        sq3 = np.broadcast_to(sq, (3, PTS)).astype(np.float16)
        m["sws24"] = np.ascontiguousarray(
            sq3.reshape(3, 8, 500).transpose(1, 0, 2).reshape(24, 500))
        for i in range(NITER):
            for b in range(BL):
                coords = np.concatenate(
                    [tgts[i][c * BL + b].astype(np.float64), sent], 0)
                rh = coords.astype(np.float16)
                rl = (coords - rh.astype(np.float64)).astype(np.float16)
                rsq = (coords ** 2).sum(1)
                rsqh = rsq.astype(np.float16)
                rsql = (rsq - rsqh.astype(np.float64)).astype(np.float16)
                R = np.zeros((128, RP), np.float16)
                for st in (0, 32):
                    R[st + 0:st + 3] = rh.T
                    R[st + 3:st + 6] = rh.T
                    R[st + 6:st + 9] = rl.T
                    R[st + 9] = rsqh
                    R[st + 10] = rsql
                m[f"rknn_{i}_{b}"] = R
                R4 = np.empty((4, RP), np.float16)
                R4[0:3] = coords.T
                R4[3] = rsq
                m[f"r4_{i}_{b}"] = R4
        in_maps.append(m)
    return in_maps


def kernel(**inputs):
    if "nc" not in _NC_CACHE:
        _NC_CACHE["nc"] = _build()
    nc = _NC_CACHE["nc"]
    in_maps = _host_prep(inputs)
    res = run_bass_kernel_spmd(nc, in_maps, list(range(NCORES))).results
    total = 0.0
    for c in range(NCORES):
        total += float(res[c]["lossT"].sum())
        total += float(res[c]["loss24"].sum())
    return np.asarray(total / B, dtype=np.float32)


# revision 36
# speedup vs baseline: 1.1572x; 1.0995x over previous
"""DenoiseNet loss kernel for 8 Trainium2 NeuronCores.

Data parallel over batch (4/core). Exact global BatchNorm via per-layer
AllReduce of (sum, sumsq); BN+ReLU applied as one affine relu(a*h+c).
Big MLP layers run fp8 DoubleRow (2 contract rows/cycle). KNN d'=2x.r-|r|^2
via fp16 hi/lo matmuls row-packed 2-wide with tile_position; argmax by
reduce-max; loss extracted by a one-hot contraction matmul
T=sum_q onehot*sw*[-2x_new^T;1] then sum_r T*[r;|r|^2]. KNN work fills
the AllReduce latency gaps.
"""
import numpy as np

import concourse.bass as bass
import concourse.mybir as mybir
import concourse.tile as tile
from concourse import bacc
from concourse.bass_utils import run_bass_kernel_spmd

dt = mybir.dt
F32 = dt.float32
F16 = dt.float16
F8 = dt.float8e4
AF = mybir.ActivationFunctionType
OP = mybir.AluOpType
AX = mybir.AxisListType
PM = mybir.MatmulPerfMode

B, N, NCORES = 32, 1000, 8
BL = B // NCORES            # 4 batches per core
PTS = BL * N                # 4000 points per core
NITER = 4
NPTS_GLOBAL = B * N         # 32000 (BN population)
EPS = 1e-5
NOISE_DECAY = 4.0
QT = 125                    # q tile (8 per batch)
RP = 1024                   # padded ref points (24 sentinels)
PT = 500                    # pts tile for MLP
NPT = PTS // PT             # 8

LAYERS = [(3, 64, 1), (64, 128, 1), (128, 256, 1), (256, 512, 1),
          (512, 1024, 1), (1024, 512, 1), (512, 256, 1), (256, 3, 0)]
NCI = [max(1, ci // 128) for ci, co, _ in LAYERS]
NCO = [max(1, (co + 127) // 128) for ci, co, _ in LAYERS]

RG = [list(range(NCORES))]
FP8L = {3, 4, 5, 6, 7}   # layers whose matmuls run fp8 DoubleRow
SPLIT_AR = {4, 5}        # layers whose stats AllReduce is split in halves
OP8 = {2, 3, 4, 5, 6}    # layers whose output is stored as fp8 pair-tiles

_NC_CACHE = {}


def _build():
    nc = bacc.Bacc(None, target_bir_lowering=False, debug=False)

    x0_d = nc.dram_tensor("x0s", [24, 500], F32, kind="ExternalInput")
    sw_d = nc.dram_tensor("sw", [128, 32], F32, kind="ExternalInput")
    sws_d = nc.dram_tensor("sws24", [24, 500], F16, kind="ExternalInput")
    id_d = nc.dram_tensor("ident8", [8, 8], F16, kind="ExternalInput")
    cneg_d = nc.dram_tensor("cneg", [2, PTS], F16, kind="ExternalInput")
    db3_d = nc.dram_tensor("db3t", [3, NITER], F32, kind="ExternalInput")
    r_d = [[nc.dram_tensor(f"rknn_{i}_{b}", [128, RP], F16,
                           kind="ExternalInput")
            for b in range(BL)] for i in range(NITER)]
    r4_d = [[nc.dram_tensor(f"r4_{i}_{b}", [4, RP], F16, kind="ExternalInput")
             for b in range(BL)] for i in range(NITER)]
    w_d = [[(nc.dram_tensor(f"w_{i}_{l}",
                            [128, NCI[l] // 2, 2,
                             16 if l == 7 else LAYERS[l][1]], F8,
                            kind="ExternalInput") if l in FP8L else
             nc.dram_tensor(f"w_{i}_{l}", list(LAYERS[l][:2]), F16,
                            kind="ExternalInput")) for l in range(8)]
           for i in range(NITER)]
    gb_d = [[nc.dram_tensor(f"gb_{i}_{l}", [128, 2, NCO[l]], F32,
                            kind="ExternalInput") for l in range(7)]
            for i in range(NITER)]
    lossT_d = nc.dram_tensor("lossT", [4, NITER * BL], F32,
                             kind="ExternalOutput")
    loss24_d = nc.dram_tensor("loss24", [24, NITER], F32,
                              kind="ExternalOutput")

    with tile.TileContext(nc) as tc:
        with (
            tc.tile_pool(name="sb", bufs=1) as sb,
            tc.tile_pool(name="ps", bufs=1, space="PSUM") as ps,
            tc.tile_pool(name="dram", bufs=2, space="DRAM") as dram,
        ):
            # ---------- persistent setup ----------
            sw_sb = sb.tile([128, 32], F32, tag="sw")
            nc.sync.dma_start(sw_sb[:], sw_d[:])
            sws24 = sb.tile([24, 500], F16, tag="sws24")
            nc.sync.dma_start(sws24[:], sws_d[:])
            ident = sb.tile([8, 8], F16, tag="ident")
            nc.sync.dma_start(ident[:], id_d[:])
            db3_sb = sb.tile([3, NITER], F32, tag="db3")
            nc.sync.dma_start(db3_sb[:], db3_d[:])
            eps_sb = sb.tile([128, 1], F32, tag="epsc")
            nc.vector.memset(eps_sb[:], float(EPS))

            x24 = sb.tile([24, 500], F32, tag="x24")
            nc.sync.dma_start(x24[:], x0_d[:])
            xf16 = sb.tile([3, NPT, PT], F16, tag="xf")

            # Ld tiles (2, by iter parity), rows replicated at strips 0/32
            Ld2 = [sb.tile([128, PTS], F16, tag=f"Ld{p}", name=f"Ld_{p}")
                   for p in range(2)]
            for p in range(2):
                nc.sync.dma_start(Ld2[p][9:11, :], cneg_d[:])
            m_all2 = [sb.tile([128, 32], F32, tag=f"ma{p}", name=f"ma_{p}")
                      for p in range(2)]
            u4 = sb.tile([128, 32, 4], F16, tag="u4")
            nc.vector.memset(u4[:, :, 3:4], 1.0)

            lossT = sb.tile([4, NITER * BL], F32, tag="lossT")
            nc.vector.memset(lossT[:], 0.0)
            loss24 = sb.tile([24, NITER], F32, tag="loss24")
            nc.vector.memset(loss24[:], 0.0)

            def load_r(it):
                rs, r4s = [], []
                for b in range(BL):
                    rt_ = sb.tile([128, RP], F16, tag=f"r{b}", bufs=2,
                                  name=f"r_{it}_{b}")
                    nc.sync.dma_start(rt_[:], r_d[it][b][:])
                    r4t = sb.tile([4, RP], F16, tag=f"r4_{b}", bufs=2,
                                  name=f"r4t_{it}_{b}")
                    nc.sync.dma_start(r4t[:], r4_d[it][b][:])
                    rs.append(rt_)
                    r4s.append(r4t)
                return rs, r4s

            def refresh_xf16():
                for p in range(NPT):
                    nc.gpsimd.dma_start(xf16[:, p, :],
                                        x24[3 * p:3 * p + 3, :])

            def build_ld(it):
                Ld = Ld2[it % 2]
                qh = sb.tile([24, 500], F16, tag="qh24", bufs=2,
                             name=f"qh_{it}")
                ql = sb.tile([24, 500], F16, tag="ql24", bufs=2,
                             name=f"ql_{it}")
                nc.vector.tensor_scalar_mul(qh[:], x24[:], 2.0)
                nc.vector.scalar_tensor_tensor(ql[:], x24[:], 2.0, qh[:],
                                               OP.mult, OP.subtract)
                for p in range(8):
                    sl = slice(p * 500, (p + 1) * 500)
                    nc.sync.dma_start(Ld[0:3, sl], qh[3 * p:3 * p + 3, :])
                    nc.sync.dma_start(Ld[3:6, sl], ql[3 * p:3 * p + 3, :])
                    nc.sync.dma_start(Ld[6:9, sl], qh[3 * p:3 * p + 3, :])
                # replicate rows 0:11 to strip at partition 32
                nc.sync.dma_start(Ld[32:43, :], Ld[0:11, :])

            # ---- phase 1: d-matmul (2-way row-packed) + row max ----
            def phase1_units(it, r_sb):
                Ld, m_all = Ld2[it % 2], m_all2[it % 2]
                for b in range(BL):
                    for qt in range(8):
                        col = b * 8 + qt
                        qsl = slice(b * N + qt * QT, b * N + (qt + 1) * QT)
                        kpd = ps.tile([QT, 2, 512], F32, tag="kpd", bufs=1,
                                      name=f"kpd1_{it}_{col}")
                        for rt in range(2):
                            st = 32 * rt
                            nc.tensor.matmul(
                                kpd[:, rt, :], Ld[st:st + 11, qsl],
                                r_sb[b][st:st + 11,
                                        rt * 512:(rt + 1) * 512],
                                start=True, stop=True,
                                tile_position=(st, 0))
                        nc.vector.tensor_reduce(m_all[0:QT, col:col + 1],
                                                kpd[:], AX.XY, OP.max)
                        yield

            # ---------- main loop ----------
            r_cur = load_r(0)
            refresh_xf16()
            build_ld(0)

            prevC = None      # generator for phase C of previous iter

            def make_phaseC(it, r_sb, r4_sb):
                Ld, m_all = Ld2[it % 2], m_all2[it % 2]

                def gen():
                    # u4 build: 8 transposes + 1 ts per batch
                    for b in range(BL):
                        xT = ps.tile([QT, 8, 4], F16, tag="Tb", bufs=1,
                                     name=f"xT_{it}_{b}")
                        for qt in range(8):
                            qs = b * N + qt * QT
                            nc.tensor.transpose(
                                xT[:, qt, 0:3],
                                xf16[0:3, (qs // PT), (qs % PT):(qs % PT) + QT],
                                ident[0:3, 0:3])
                        nc.vector.tensor_scalar_mul(
                            u4[0:QT, 8 * b:8 * b + 8, 0:3],
                            xT[:, :, 0:3], -2.0)
                        yield
                    # per (b,qt): recompute d', one-hot*sw, contract
                    for b in range(BL):
                        Tb = ps.tile([4, RP], F32, tag="Tb", bufs=1,
                                     name=f"Tb_{it}_{b}")
                        for qt in range(8):
                            col = b * 8 + qt
                            qsl = slice(b * N + qt * QT,
                                        b * N + (qt + 1) * QT)
                            kpd = ps.tile([QT, 2, 512], F32, tag="kpd",
                                          bufs=1, name=f"kpdC_{it}_{col}")
                            for rt in range(2):
                                st = 32 * rt
                                nc.tensor.matmul(
                                    kpd[:, rt, :], Ld[st:st + 11, qsl],
                                    r_sb[b][st:st + 11,
                                            rt * 512:(rt + 1) * 512],
                                    start=True, stop=True,
                                    tile_position=(st, 0))
                            oh = sb.tile([QT, 2, 512], F16, tag="oh", bufs=2,
                                         name=f"oh_{it}_{col}")
                            nc.vector.tensor_scalar(
                                oh[:], kpd[:], m_all[0:QT, col:col + 1],
                                sw_sb[0:QT, col:col + 1],
                                OP.is_equal, OP.mult)
                            for rt in range(2):
                                nc.tensor.matmul(
                                    Tb[:, rt * 512:(rt + 1) * 512],
                                    u4[0:QT, col, :], oh[:, rt, :],
                                    start=(qt == 0), stop=(qt == 7))
                            yield
                        jk4 = sb.tile([4, RP], F16, tag="jk4", bufs=2,
                                      name=f"jk4_{it}_{b}")
                        nc.vector.scalar_tensor_tensor(
                            jk4[:], Tb[:], 1.0, r4_sb[b][:],
                            OP.mult, OP.mult,
                            accum_out=lossT[0:4, BL * it + b:BL * it + b + 1])
                        yield
                    # term1: sum sw*|x_new|^2
                    wv = sb.tile([24, 500], F16, tag="wv", bufs=2,
                                 name=f"wv_{it}")
                    nc.vector.tensor_tensor(out=wv[:], in0=x24[:],
                                            in1=sws24[:], op=OP.mult)
                    jkw = sb.tile([24, 500], F16, tag="jkw", bufs=2,
                                  name=f"jkw_{it}")
                    nc.vector.scalar_tensor_tensor(
                        jkw[:], wv[:], 1.0, wv[:], OP.mult, OP.mult,
                        accum_out=loss24[0:24, it:it + 1])
                    yield
                return gen()

            def drain(gen, n):
                if gen is None:
                    return
                for _ in range(n):
                    try:
                        next(gen)
                    except StopIteration:
                        break

            for it in range(NITER):
                Ld, m_all = Ld2[it % 2], m_all2[it % 2]
                r_sb, r4_sb = r_cur
                if it + 1 < NITER:
                    r_nxt = load_r(it + 1)

                p1 = phase1_units(it, r_sb)

                # ---------- MLP ----------
                rhs, rhs8 = None, None
                preds3 = sb.tile([3, NPT, PT], F16, tag="preds3",
                                 name=f"preds3_{it}")
                preds24 = sb.tile([24, 500], F32, tag="preds",
                                  name=f"preds_{it}")

                for l in range(8):
                    cin, cout, has_bn = LAYERS[l]
                    nci, nco = NCI[l], NCO[l]
                    CIP = min(128, cin)

                    if has_bn:
                        gb = sb.tile([128, 2, nco], F32, tag="gb", bufs=2,
                                     name=f"gb_{it}_{l}")
                        nc.sync.dma_start(gb[:], gb_d[it][l][:])
                        sums = sb.tile([128, nco, NPT], F32, tag="sums",
                                       bufs=2, name=f"sums_{it}_{l}")
                        statsr = sb.tile([128, 2, nco], F32, tag="statsr",
                                         bufs=2, name=f"statsr_{it}_{l}")
                        if l in OP8:
                            zt8 = [sb.tile([128, 2, PTS], F8,
                                           tag=f"y{l % 2}_{p}",
                                           name=f"y_{it}_{l}_{p}")
                                   for p in range(nco // 2)]
                            zt = None
                        else:
                            zt = [sb.tile([128, PTS], F16,
                                          tag=f"z{l % 2}_{co}",
                                          name=f"z_{it}_{l}_{co}")
                                  for co in range(nco)]
                            zt8 = None

                    def zsl(co, CO, colsl):
                        if zt8 is not None:
                            return zt8[co // 2][0:CO, co % 2, colsl]
                        return zt[co][0:CO, colsl]

                    for co in range(nco):
                        CO = min(128, cout - co * 128)
                        COP = 16 if l == 7 else CO
                        wr = []
                        if l in FP8L:
                            for p in range(nci // 2):
                                w = sb.tile([128, 2, COP], F8,
                                            tag=f"wc{co % 2}_{p}", bufs=2,
                                            name=f"w_{it}_{l}_{co}_{p}")
                                nc.sync.dma_start(
                                    w[:], w_d[it][l][:, p, :,
                                                     co * 128:co * 128 + COP])
                                wr.append(w)
                        else:
                            for ci in range(nci):
                                w = sb.tile([CIP, CO], F16,
                                            tag=f"wc{co % 2}_{ci}", bufs=2,
                                            name=f"w_{it}_{l}_{co}_{ci}")
                                nc.sync.dma_start(
                                    w[:],
                                    w_d[it][l][ci * 128:ci * 128 + CIP,
                                               co * 128:co * 128 + CO])
                                wr.append(w)
                        for pt in range(NPT):
                            ptsl = slice(pt * PT, (pt + 1) * PT)
                            zp = ps.tile([128, PT], F32, tag="zp", bufs=4,
                                         name=f"zp_{it}_{l}_{co}_{pt}")
                            if l in FP8L:
                                for p in range(nci // 2):
                                    nc.tensor.matmul(
                                        zp[0:COP, :], wr[p][:],
                                        rhs8[p][:, :, ptsl],
                                        start=(p == 0),
                                        stop=(p == nci // 2 - 1),
                                        perf_mode=PM.DoubleRow)
                            else:
                                for ci in range(nci):
                                    if l == 0:
                                        rv = xf16[:, pt, :]
                                    else:
                                        rv = rhs[ci][0:CIP, ptsl]
                                    nc.tensor.matmul(
                                        zp[0:CO, :], wr[ci][:], rv,
                                        start=(ci == 0),
                                        stop=(ci == nci - 1))
                            if has_bn:
                                if (co + pt) % 2 == 0:
                                    nc.scalar.activation(
                                        zsl(co, CO, ptsl), zp[0:CO, :],
                                        AF.Copy,
                                        accum_out=sums[0:CO, co, pt:pt + 1])
                                else:
                                    nc.vector.tensor_scalar(
                                        zsl(co, CO, ptsl), zp[0:CO, :],
                                        1.0, 0.0, OP.mult, OP.add,
                                        accum_out=sums[0:CO, co, pt:pt + 1])
                            else:
                                nc.scalar.activation(
                                    preds3[:, pt, :], zp[0:3, :], AF.Tanh,
                                    bias=db3_sb[:, it:it + 1])
                        if has_bn:
                            # sumsq: even chunks on ACT (Square), odd on DVE
                            jk = sb.tile([128, PTS], F16, tag="jksq", bufs=1,
                                         name=f"jksq_{it}_{l}_{co}")
                            if co < nco - 1:
                                if co % 2 == 0:
                                    nc.scalar.activation(
                                        jk[0:CO, :],
                                        zsl(co, CO, slice(0, PTS)),
                                        AF.Square,
                                        accum_out=statsr[0:CO, 1, co:co + 1])
                                else:
                                    nc.vector.scalar_tensor_tensor(
                                        jk[0:CO, :],
                                        zsl(co, CO, slice(0, PTS)), 1.0,
                                        zsl(co, CO, slice(0, PTS)),
                                        OP.mult, OP.mult,
                                        accum_out=statsr[0:CO, 1, co:co + 1])
                            else:
                                sq2 = sb.tile([128, 2], F32, tag="sq2",
                                              bufs=2, name=f"sq2_{it}_{l}")
                                H = PTS // 2
                                for hh in range(2):
                                    eng = nc.scalar if hh == 0 else nc.vector
                                    if hh == 0:
                                        nc.scalar.activation(
                                            jk[0:CO, 0:H],
                                            zsl(co, CO, slice(0, H)),
                                            AF.Square,
                                            accum_out=sq2[0:CO, 0:1])
                                    else:
                                        nc.vector.scalar_tensor_tensor(
                                            jk[0:CO, H:PTS],
                                            zsl(co, CO, slice(H, PTS)), 1.0,
                                            zsl(co, CO, slice(H, PTS)),
                                            OP.mult, OP.mult,
                                            accum_out=sq2[0:CO, 1:2])
                                nc.vector.tensor_reduce(
                                    statsr[0:CO, 1, co:co + 1],
                                    sq2[0:CO, :], AX.X, OP.add)

                    if not has_bn:
                        break

                    nc.vector.tensor_reduce(statsr[:, 0, :], sums[:], AX.X,
                                            OP.add)
                    arin = dram.tile([128, 2, nco], F32, tag="arin")
                    arout = dram.tile([128, 2, nco], F32, tag="arout")
                    nc.sync.dma_start(arin[:], statsr[:])
                    nc.gpsimd.collective_compute(
                        "AllReduce", OP.add, replica_groups=RG,
                        ins=[arin.opt()], outs=[arout.opt()])
                    statsg = sb.tile([128, 2, nco], F32, tag="statsg", bufs=2,
                                     name=f"statsg_{it}_{l}")
                    nc.sync.dma_start(statsg[:], arout[:])

                    if l <= 3:
                        drain(prevC, 15)
                    else:
                        drain(prevC, 99)
                        drain(p1, 12 if l in (4, 5) else 99)

                    af = sb.tile([128, 6, nco], F32, tag="af", bufs=2,
                                 name=f"af_{it}_{l}")
                    inv_n = 1.0 / NPTS_GLOBAL
                    nc.vector.tensor_scalar_mul(af[:, 2, :], statsg[:, 0, :],
                                                inv_n)  # m
                    nc.vector.scalar_tensor_tensor(
                        af[:, 3, :], statsg[:, 0, :], inv_n,
                        af[:, 2, :], OP.mult, OP.mult)  # m^2
                    nc.vector.scalar_tensor_tensor(
                        af[:, 3, :], statsg[:, 1, :], inv_n,
                        af[:, 3, :], OP.mult, OP.subtract)  # v
                    nc.scalar.activation(af[:, 4, :], af[:, 3, :], AF.Sqrt,
                                         bias=eps_sb[:])
                    nc.vector.reciprocal(af[:, 5, :], af[:, 4, :])
                    nc.vector.tensor_tensor(out=af[:, 0, :], in0=gb[:, 0, :],
                                            in1=af[:, 5, :], op=OP.mult)
                    nc.vector.tensor_tensor(out=af[:, 4, :], in0=af[:, 2, :],
                                            in1=af[:, 0, :], op=OP.mult)
                    nc.vector.tensor_tensor(out=af[:, 1, :], in0=gb[:, 1, :],
                                            in1=af[:, 4, :], op=OP.subtract)

                    def _aff(co, CO, col, eng):
                        dst = zsl(co, CO, col)
                        if eng == 0:
                            nc.scalar.activation(
                                dst, dst, AF.Relu,
                                bias=af[0:CO, 1, co:co + 1],
                                scale=af[0:CO, 0, co:co + 1])
                        else:
                            nc.vector.tensor_scalar(
                                dst, dst,
                                af[0:CO, 0, co:co + 1],
                                af[0:CO, 1, co:co + 1], OP.mult, OP.add)
                            nc.vector.tensor_scalar_max(dst, dst, 0.0)

                    # --- affine+relu: per pt tile, alternating engines ---
                    for sl4 in range(4):
                        for co in range(nco):
                            CO = min(128, cout - co * 128)
                            _aff(co, CO, slice(sl4 * 1000, (sl4 + 1) * 1000),
                                 (co + sl4) % 2)
                    rhs, rhs8 = zt, zt8

                # ---------- x update; spill leftover filler ----------
                drain(prevC, 99)
                drain(p1, 99)
                for p in range(NPT):
                    nc.gpsimd.dma_start(preds24[3 * p:3 * p + 3, :],
                                        preds3[:, p, :])
                nc.vector.tensor_tensor(out=x24[:], in0=x24[:],
                                        in1=preds24[:], op=OP.add)
                refresh_xf16()
                if it + 1 < NITER:
                    build_ld(it + 1)
                prevC = make_phaseC(it, r_sb, r4_sb)
                if it + 1 < NITER:
                    r_cur = r_nxt

            # tail: phase C of last iter
            drain(prevC, 999)

            nc.sync.dma_start(lossT_d[:], lossT[:])
            nc.sync.dma_start(loss24_d[:], loss24[:])
    nc.compile()
    return nc


def _host_prep(inputs):
    f32 = np.float32
    noisy = np.asarray(inputs["pcl_noisy"], f32)
    clean = np.asarray(inputs["pcl_clean"], f32)
    seeds = np.asarray(inputs["pcl_seeds"], f32)
    std = np.asarray(inputs["pcl_std"], f32)
    noise = np.asarray(inputs["noise"], f32)

    pn = noisy - seeds
    pc = clean - seeds
    sdist = np.sum(pn.astype(np.float64) ** 2, -1, keepdims=True)
    max_sq = sdist[:, -1:, :]
    sw = np.exp(-sdist * 9.0 / max_sq)[..., 0]
    sw = (sw / sw.sum(1, keepdims=True))  # [B, N] float64

    tgts = []
    cur = std.copy()
    for i in range(NITER):
        if i < NITER - 1:
            cur = cur / NOISE_DECAY
            tgts.append(pc + noise[i] * cur[:, None, None])
        else:
            tgts.append(pc.copy())

    sent = np.full((RP - N, 3), 100.0, np.float64)

    shared = {}
    f8np = mybir.dt.np(F8)
    for i in range(NITER):
        for l in range(8):
            key = f'ew{l+1}' if l < 5 else f'dw{l-4}'
            W = np.asarray(inputs[key], f32)[i]
            if l in FP8L:
                P2 = NCI[l] // 2
                cw = LAYERS[l][1]
                if l == 7:
                    Wp = np.zeros((W.shape[0], 16), f32)
                    Wp[:, :cw] = W
                    W, cw = Wp, 16
                arr = W.reshape(P2, 2, 128, cw).transpose(2, 0, 1, 3)
                shared[f"w_{i}_{l}"] = np.ascontiguousarray(arr).astype(f8np)
            else:
                shared[f"w_{i}_{l}"] = W.astype(np.float16)
        for l in range(7):
            nco = NCO[l]
            cout = LAYERS[l][1]
            gk = f'eg{l+1}' if l < 5 else f'dg{l-4}'
            hk = f'eh{l+1}' if l < 5 else f'dh{l-4}'
            g = np.asarray(inputs[gk], f32)[i]
            h = np.asarray(inputs[hk], f32)[i]
            arr = np.zeros((128, 2, nco), f32)
            gp = np.zeros(nco * 128, f32); gp[:cout] = g
            hp = np.zeros(nco * 128, f32); hp[:cout] = h
            arr[:, 0, :] = gp.reshape(nco, 128).T
            arr[:, 1, :] = hp.reshape(nco, 128).T
            shared[f"gb_{i}_{l}"] = arr
    shared["db3t"] = np.ascontiguousarray(np.asarray(inputs["db3"], f32).T)
    shared["ident8"] = np.eye(8, dtype=np.float16)
    shared["cneg"] = np.full((2, PTS), -1.0, np.float16)

    in_maps = []
    for c in range(NCORES):
        bs = slice(c * BL, (c + 1) * BL)
        m = dict(shared)
        x3 = pn[bs].transpose(2, 0, 1).reshape(3, PTS)  # [3, 4000]
        m["x0s"] = np.ascontiguousarray(
            x3.reshape(3, 8, 500).transpose(1, 0, 2).reshape(24, 500))
        swc = np.zeros((128, 32), f32)
        for b in range(BL):
            for qt in range(8):
                swc[0:QT, b * 8 + qt] = sw[c * BL + b,
                                           qt * QT:(qt + 1) * QT].astype(f32)
        m["sw"] = swc
        sq = np.sqrt(sw[bs]).reshape(1, PTS)
        sq3 = np.broadcast_to(sq, (3, PTS)).astype(np.float16)
        m["sws24"] = np.ascontiguousarray(
            sq3.reshape(3, 8, 500).transpose(1, 0, 2).reshape(24, 500))
        for i in range(NITER):
            for b in range(BL):
                coords = np.concatenate(
                    [tgts[i][c * BL + b].astype(np.float64), sent], 0)
                rh = coords.astype(np.float16)
                rl = (coords - rh.astype(np.float64)).astype(np.float16)
                rsq = (coords ** 2).sum(1)
                rsqh = rsq.astype(np.float16)
                rsql = (rsq - rsqh.astype(np.float64)).astype(np.float16)
                R = np.zeros((128, RP), np.float16)
                for st in (0, 32):
                    R[st + 0:st + 3] = rh.T
                    R[st + 3:st + 6] = rh.T
                    R[st + 6:st + 9] = rl.T
                    R[st + 9] = rsqh
                    R[st + 10] = rsql
                m[f"rknn_{i}_{b}"] = R
                R4 = np.empty((4, RP), np.float16)
                R4[0:3] = coords.T
                R4[3] = rsq
                m[f"r4_{i}_{b}"] = R4
        in_maps.append(m)
    return in_maps


def kernel(**inputs):
    if "nc" not in _NC_CACHE:
        _NC_CACHE["nc"] = _build()
    nc = _NC_CACHE["nc"]
    in_maps = _host_prep(inputs)
    res = run_bass_kernel_spmd(nc, in_maps, list(range(NCORES))).results
    total = 0.0
    for c in range(NCORES):
        total += float(res[c]["lossT"].sum())
        total += float(res[c]["loss24"].sum())
    return np.asarray(total / B, dtype=np.float32)
